# revision 10
# baseline (speedup 1.0000x reference)
"""MoE (8 experts, top-2) Trainium2 kernel.

Strategy (expert-parallel, per the sharding hint):
  - Host computes the tiny router (logits/softmax/top-2/gates + aux loss).
  - Each of the 8 NeuronCores gets one expert: the tokens routed to that
    expert are gathered into a transposed [D, C] activation block (C =
    max expert load, padded), plus that expert's W1/b1/W2/b2.
  - On-core: H^T = gelu(W1^T @ X^T + b1), Y^T = W2^T @ H^T + b2, both as
    chains of 128x128xN matmuls that keep the contraction on the partition
    dim, so no on-device transposes are needed anywhere.
  - Host scatter-adds gate * Y back into the full output (exact: non-top-2
    experts have combine weight 0 in the reference).

Shapes are hardcoded for the graded problem: x [1,2048,1024], E=8, D=1024,
H=4096, top-2. The Bass program is built after routing is known, so the
capacity C adapts to the actual input.
"""

import numpy as np
import ml_dtypes

DIM = 1024
HIDDEN = 4096
NUM_EXPERTS = 8
TOP_K = 2
AUX_COEF = 0.01

P = 128
KS = DIM // P      # 8   k-subtiles for MM1 contraction
HC = HIDDEN // P   # 32  h-chunks (MM1 output partitions / MM2 contraction)
DC = DIM // P      # 8   d-chunks (MM2 output partitions)

_BF16 = ml_dtypes.bfloat16

_nc_cache = {}


def _build_bass(C, chunks):
    import concourse.tile as tile
    from concourse import bacc, mybir

    dt = mybir.dt
    act = mybir.ActivationFunctionType

    nc = bacc.Bacc(None, target_bir_lowering=False, debug=False)

    xt_d = nc.dram_tensor("xt", [P, KS, C], dt.bfloat16, kind="ExternalInput")
    w1_d = nc.dram_tensor("w1", [HC // 2, P, KS, 2, P], dt.bfloat16, kind="ExternalInput")
    b1_d = nc.dram_tensor("b1", [P, HC], dt.float32, kind="ExternalInput")
    w2_d = nc.dram_tensor("w2", [DC, P, HC, P], dt.bfloat16, kind="ExternalInput")
    b2_d = nc.dram_tensor("b2", [P, DC], dt.float32, kind="ExternalInput")
    yt_d = nc.dram_tensor("yt", [DC, P, C], dt.float32, kind="ExternalOutput")

    with tile.TileContext(nc) as tc:
        with (
            tc.tile_pool(name="big", bufs=1) as big,
            tc.tile_pool(name="w1p", bufs=4) as w1p,
            tc.tile_pool(name="w2p", bufs=4) as w2p,
            tc.tile_pool(name="yp", bufs=3) as yp,
            tc.tile_pool(name="ps", bufs=6, space="PSUM") as psp,
            tc.tile_pool(name="wps", bufs=1, space="PSUM") as wpsp,
        ):
            # PE warmup: dummy matmuls on a zeroed tile while input DMAs
            # are in flight, so the HAM clock-gate is already at 2.4 GHz
            # when the first real matmul issues (saves the ~3.4us cold
            # window at half rate).
            warm = big.tile([P, 256], dt.bfloat16)
            nc.any.memzero(warm[:])
            wps = wpsp.tile([P, 512], dt.float32)
            for _ in range(20):
                nc.tensor.matmul(wps[:, :256], warm[:, :P], warm[:])

            # xt + weights all on the sync HWDGE ring in exact consumption
            # order (queue is FIFO; packets of different queues round-robin,
            # so the critical path must be alone at the head of one queue).
            # The first real matmul needs only w1[hc=0] + xt chunk 0.
            xt_s = big.tile([P, KS, C], dt.bfloat16)
            w1_singles = []
            w1_singles.append(w1p.tile([P, KS, P], dt.bfloat16, tag="w1s", name="w1s"))
            nc.sync.dma_start(w1_singles[0][:], w1_d[0][:, :, 0])
            for (t0, t1) in chunks:
                for ks in range(0, KS, 2):
                    nc.sync.dma_start(
                        xt_s[:, ks : ks + 2, t0:t1], xt_d[:, ks : ks + 2, t0:t1]
                    )
            for hc in range(1, 4):
                w1t = w1p.tile([P, KS, P], dt.bfloat16, tag="w1s", name="w1s")
                nc.sync.dma_start(w1t[:], w1_d[hc // 2][:, :, hc % 2])
                w1_singles.append(w1t)
            b1_s = big.tile([P, HC], dt.float32)
            nc.scalar.dma_start(b1_s[:], b1_d[:])
            b2_s = big.tile([P, DC], dt.float32)
            nc.scalar.dma_start(b2_s[:], b2_d[:])
            h_s = big.tile([P, HC, C], dt.bfloat16)

            # MM1: H^T[h, t] = gelu(sum_d W1[d, h] * X^T[d, t] + b1[h])
            def mm1(hc, w1_lhsT):
                for (t0, t1) in chunks:
                    n = t1 - t0
                    ps = psp.tile([P, 512], dt.float32, tag="ps", name="ps")[:, :n]
                    for ks in range(KS):
                        nc.tensor.matmul(
                            ps,
                            w1_lhsT(ks),
                            xt_s[:, ks, t0:t1],
                            start=(ks == 0),
                            stop=(ks == KS - 1),
                        )
                    nc.scalar.activation(
                        h_s[:, hc, t0:t1], ps, act.Gelu, bias=b1_s[:, hc : hc + 1]
                    )

            for hc in range(4):
                mm1(hc, lambda ks, t=w1_singles[hc]: t[:, ks])
            for hc2 in range(2, HC // 2):
                w1_t = w1p.tile([P, KS, 2, P], dt.bfloat16)
                nc.sync.dma_start(w1_t[:], w1_d[hc2])
                for half in range(2):
                    mm1(2 * hc2 + half, lambda ks, t=w1_t, h=half: t[:, ks, h])

            # MM2: Y^T[d, t] = sum_h W2[h, d] * H^T[h, t] + b2[d]
            for dc in range(DC):
                w2_t = w2p.tile([P, HC, P], dt.bfloat16)
                nc.sync.dma_start(w2_t[:], w2_d[dc])
                y_t = yp.tile([P, C], dt.float32, tag="y", name="y")
                for (t0, t1) in chunks:
                    n = t1 - t0
                    ps = psp.tile([P, 512], dt.float32, tag="ps", name="ps")[:, :n]
                    for hs in range(HC):
                        nc.tensor.matmul(
                            ps,
                            w2_t[:, hs],
                            h_s[:, hs, t0:t1],
                            start=(hs == 0),
                            stop=(hs == HC - 1),
                        )
                    nc.scalar.activation(
                        y_t[:, t0:t1], ps, act.Identity, bias=b2_s[:, dc : dc + 1]
                    )
                nc.scalar.dma_start(yt_d[dc], y_t[:])

    nc.compile()
    return nc


def _route(x, router_w):
    """Replicates the reference router in fp32 numpy."""
    xt = x.reshape(-1, DIM).astype(np.float32)
    logits = xt @ router_w.T.astype(np.float32)
    m = logits.max(-1, keepdims=True)
    e = np.exp(logits - m)
    probs = e / e.sum(-1, keepdims=True)
    # top-2, ties -> lower index first (matches jax.lax.top_k)
    idx = np.argsort(-probs, axis=-1, kind="stable")[:, :TOP_K]
    gates = np.take_along_axis(probs, idx, axis=-1)
    gates = gates / (gates.sum(-1, keepdims=True) + 1e-9)
    # aux loss
    counts = np.zeros(NUM_EXPERTS, np.float64)
    for k in range(TOP_K):
        counts += np.bincount(idx[:, k], minlength=NUM_EXPERTS)
    onehot_mean = (counts / (idx.shape[0] * TOP_K)).astype(np.float32)
    aux = np.float32(AUX_COEF * NUM_EXPERTS * (onehot_mean * probs.mean(0)).sum())
    return xt, idx, gates, aux


def kernel(x, router_w, W1, b1, W2, b2):
    from concourse.bass_utils import run_bass_kernel_spmd

    xt, idx, gates, aux = _route(x, router_w)
    T = xt.shape[0]

    # per-expert token lists and combine weights
    tok, wgt = [], []
    for e in range(NUM_EXPERTS):
        sel0 = idx[:, 0] == e
        sel1 = idx[:, 1] == e
        t = np.flatnonzero(sel0 | sel1)
        w = np.where(sel0[t], gates[t, 0], gates[t, 1]).astype(np.float32)
        tok.append(t)
        wgt.append(w)

    maxload = max(len(t) for t in tok)
    n_chunks = max(1, -(-maxload // 512))
    csz = -(-(-(-maxload // n_chunks)) // 16) * 16  # chunk size, mult of 16
    C = csz * n_chunks
    chunks = tuple((i * csz, (i + 1) * csz) for i in range(n_chunks))

    key = (C, chunks)
    if key not in _nc_cache:
        _nc_cache[key] = _build_bass(C, chunks)
    nc = _nc_cache[key]

    # per-core input maps, weights pre-packed into the exact SBUF layouts
    W1 = np.asarray(W1, np.float32)
    W2 = np.asarray(W2, np.float32)
    b1 = np.asarray(b1, np.float32)
    b2 = np.asarray(b2, np.float32)
    in_maps = []
    for e in range(NUM_EXPERTS):
        xe = np.zeros((C, DIM), np.float32)
        xe[: len(tok[e])] = xt[tok[e]]
        # [C, D] -> [P, KS, C] with d = ks*P + p
        xt_pack = np.ascontiguousarray(
            xe.T.reshape(KS, P, C).transpose(1, 0, 2)
        ).astype(_BF16)
        # W1[d, h] -> [HC//2, P, KS, 2, P] : [hc2, p_d, ks, half, p_h]
        w1_pack = np.ascontiguousarray(
            W1[e].reshape(KS, P, HC // 2, 2, P).transpose(2, 1, 0, 3, 4)
        ).astype(_BF16)
        # W2[h, d] -> [DC, P, HC, P] : [dc, p_h, hs, p_d]
        w2_pack = np.ascontiguousarray(
            W2[e].reshape(HC, P, DC, P).transpose(2, 1, 0, 3)
        ).astype(_BF16)
        b1_pack = np.ascontiguousarray(b1[e].reshape(HC, P).T)
        b2_pack = np.ascontiguousarray(b2[e].reshape(DC, P).T)
        in_maps.append(
            {
                "xt": xt_pack,
                "w1": w1_pack,
                "b1": b1_pack,
                "w2": w2_pack,
                "b2": b2_pack,
            }
        )

    global _last_in_maps
    _last_in_maps = in_maps
    res = run_bass_kernel_spmd(nc, in_maps, core_ids=list(range(NUM_EXPERTS)))

    out = np.zeros((T, DIM), np.float32)
    for e in range(NUM_EXPERTS):
        yt = np.asarray(res.results[e]["yt"], np.float32)  # [DC, P, C]
        y = yt.reshape(DIM, C)[:, : len(tok[e])].T  # [n_e, D]
        out[tok[e]] += wgt[e][:, None] * y

    return out.reshape(x.shape), aux


# revision 11
# speedup vs baseline: 1.0093x; 1.0093x over previous
"""MoE (8 experts, top-2) Trainium2 kernel.

Strategy (expert-parallel, per the sharding hint):
  - Host computes the tiny router (logits/softmax/top-2/gates + aux loss).
  - Each of the 8 NeuronCores gets one expert: the tokens routed to that
    expert are gathered into a transposed [D, C] activation block (C =
    max expert load, padded), plus that expert's W1/b1/W2/b2.
  - On-core: H^T = gelu(W1^T @ X^T + b1), Y^T = W2^T @ H^T + b2, both as
    chains of 128x128xN matmuls that keep the contraction on the partition
    dim, so no on-device transposes are needed anywhere.
  - Host scatter-adds gate * Y back into the full output (exact: non-top-2
    experts have combine weight 0 in the reference).

Shapes are hardcoded for the graded problem: x [1,2048,1024], E=8, D=1024,
H=4096, top-2. The Bass program is built after routing is known, so the
capacity C adapts to the actual input.
"""

import numpy as np
import ml_dtypes

DIM = 1024
HIDDEN = 4096
NUM_EXPERTS = 8
TOP_K = 2
AUX_COEF = 0.01

P = 128
KS = DIM // P      # 8   k-subtiles for MM1 contraction
HC = HIDDEN // P   # 32  h-chunks (MM1 output partitions / MM2 contraction)
DC = DIM // P      # 8   d-chunks (MM2 output partitions)

_BF16 = ml_dtypes.bfloat16

_nc_cache = {}


def _build_bass(C, chunks):
    import concourse.tile as tile
    from concourse import bacc, mybir

    dt = mybir.dt
    act = mybir.ActivationFunctionType

    nc = bacc.Bacc(None, target_bir_lowering=False, debug=False)

    xt_d = nc.dram_tensor("xt", [len(chunks), P, KS, chunks[0][1]], dt.bfloat16, kind="ExternalInput")
    w1_d = nc.dram_tensor("w1", [HC // 2, P, KS, 2, P], dt.bfloat16, kind="ExternalInput")
    b1_d = nc.dram_tensor("b1", [P, HC], dt.float32, kind="ExternalInput")
    w2_d = nc.dram_tensor("w2", [DC, P, HC, P], dt.bfloat16, kind="ExternalInput")
    b2_d = nc.dram_tensor("b2", [P, DC], dt.float32, kind="ExternalInput")
    yt_d = nc.dram_tensor("yt", [DC, P, C], dt.float32, kind="ExternalOutput")

    with tile.TileContext(nc) as tc:
        with (
            tc.tile_pool(name="big", bufs=1) as big,
            tc.tile_pool(name="w1p", bufs=4) as w1p,
            tc.tile_pool(name="w2p", bufs=4) as w2p,
            tc.tile_pool(name="yp", bufs=3) as yp,
            tc.tile_pool(name="ps", bufs=6, space="PSUM") as psp,
            tc.tile_pool(name="wps", bufs=1, space="PSUM") as wpsp,
        ):
            # PE warmup: dummy matmuls on a zeroed tile while input DMAs
            # are in flight, so the HAM clock-gate is already at 2.4 GHz
            # when the first real matmul issues (saves the ~3.4us cold
            # window at half rate).
            warm = big.tile([P, 256], dt.bfloat16)
            nc.any.memzero(warm[:])
            wps = wpsp.tile([P, 512], dt.float32)
            for _ in range(16):
                nc.tensor.matmul(wps[:, :256], warm[:, :P], warm[:])

            # xt + weights all on the sync HWDGE ring in exact consumption
            # order (queue is FIFO; packets of different queues round-robin,
            # so the critical path must be alone at the head of one queue).
            # The first real matmul needs only w1[hc=0] + xt chunk 0.
            xt_s = big.tile([P, KS, C], dt.bfloat16)
            w1_singles = [w1p.tile([P, KS, P], dt.bfloat16, tag="w1s", name="w1s")
                          for _ in range(4)]
            nc.sync.dma_start(w1_singles[0][:], w1_d[0][:, :, 0])
            for ci, (t0, t1) in enumerate(chunks):
                nc.sync.dma_start(xt_s[:, :, t0:t1], xt_d[ci])
                if ci + 1 < 4:
                    hc = ci + 1
                    nc.sync.dma_start(w1_singles[hc][:], w1_d[hc // 2][:, :, hc % 2])
            for hc in range(len(chunks) + 1, 4):
                nc.sync.dma_start(w1_singles[hc][:], w1_d[hc // 2][:, :, hc % 2])
            b1_s = big.tile([P, HC], dt.float32)
            nc.scalar.dma_start(b1_s[:], b1_d[:])
            b2_s = big.tile([P, DC], dt.float32)
            nc.scalar.dma_start(b2_s[:], b2_d[:])
            h_s = big.tile([P, HC, C], dt.bfloat16)

            # MM1: H^T[h, t] = gelu(sum_d W1[d, h] * X^T[d, t] + b1[h])
            def mm1(hc, w1_lhsT):
                for (t0, t1) in chunks:
                    n = t1 - t0
                    ps = psp.tile([P, 512], dt.float32, tag="ps", name="ps")[:, :n]
                    for ks in range(KS):
                        nc.tensor.matmul(
                            ps,
                            w1_lhsT(ks),
                            xt_s[:, ks, t0:t1],
                            start=(ks == 0),
                            stop=(ks == KS - 1),
                        )
                    nc.scalar.activation(
                        h_s[:, hc, t0:t1], ps, act.Gelu, bias=b1_s[:, hc : hc + 1]
                    )

            for hc in range(4):
                mm1(hc, lambda ks, t=w1_singles[hc]: t[:, ks])
            for hc2 in range(2, HC // 2):
                w1_t = w1p.tile([P, KS, 2, P], dt.bfloat16)
                nc.sync.dma_start(w1_t[:], w1_d[hc2])
                for half in range(2):
                    mm1(2 * hc2 + half, lambda ks, t=w1_t, h=half: t[:, ks, h])

            # MM2: Y^T[d, t] = sum_h W2[h, d] * H^T[h, t] + b2[d]
            for dc in range(DC):
                w2_t = w2p.tile([P, HC, P], dt.bfloat16)
                nc.sync.dma_start(w2_t[:], w2_d[dc])
                y_t = yp.tile([P, C], dt.float32, tag="y", name="y")
                for (t0, t1) in chunks:
                    n = t1 - t0
                    ps = psp.tile([P, 512], dt.float32, tag="ps", name="ps")[:, :n]
                    for hs in range(HC):
                        nc.tensor.matmul(
                            ps,
                            w2_t[:, hs],
                            h_s[:, hs, t0:t1],
                            start=(hs == 0),
                            stop=(hs == HC - 1),
                        )
                    nc.scalar.activation(
                        y_t[:, t0:t1], ps, act.Identity, bias=b2_s[:, dc : dc + 1]
                    )
                nc.scalar.dma_start(yt_d[dc], y_t[:])

    nc.compile()
    return nc


def _route(x, router_w):
    """Replicates the reference router in fp32 numpy."""
    xt = x.reshape(-1, DIM).astype(np.float32)
    logits = xt @ router_w.T.astype(np.float32)
    m = logits.max(-1, keepdims=True)
    e = np.exp(logits - m)
    probs = e / e.sum(-1, keepdims=True)
    # top-2, ties -> lower index first (matches jax.lax.top_k)
    idx = np.argsort(-probs, axis=-1, kind="stable")[:, :TOP_K]
    gates = np.take_along_axis(probs, idx, axis=-1)
    gates = gates / (gates.sum(-1, keepdims=True) + 1e-9)
    # aux loss
    counts = np.zeros(NUM_EXPERTS, np.float64)
    for k in range(TOP_K):
        counts += np.bincount(idx[:, k], minlength=NUM_EXPERTS)
    onehot_mean = (counts / (idx.shape[0] * TOP_K)).astype(np.float32)
    aux = np.float32(AUX_COEF * NUM_EXPERTS * (onehot_mean * probs.mean(0)).sum())
    return xt, idx, gates, aux


def kernel(x, router_w, W1, b1, W2, b2):
    from concourse.bass_utils import run_bass_kernel_spmd

    xt, idx, gates, aux = _route(x, router_w)
    T = xt.shape[0]

    # per-expert token lists and combine weights
    tok, wgt = [], []
    for e in range(NUM_EXPERTS):
        sel0 = idx[:, 0] == e
        sel1 = idx[:, 1] == e
        t = np.flatnonzero(sel0 | sel1)
        w = np.where(sel0[t], gates[t, 0], gates[t, 1]).astype(np.float32)
        tok.append(t)
        wgt.append(w)

    maxload = max(len(t) for t in tok)
    n_chunks = max(1, -(-maxload // 512))
    csz = -(-(-(-maxload // n_chunks)) // 16) * 16  # chunk size, mult of 16
    C = csz * n_chunks
    chunks = tuple((i * csz, (i + 1) * csz) for i in range(n_chunks))

    key = (C, chunks)
    if key not in _nc_cache:
        _nc_cache[key] = _build_bass(C, chunks)
    nc = _nc_cache[key]

    # per-core input maps, weights pre-packed into the exact SBUF layouts
    W1 = np.asarray(W1, np.float32)
    W2 = np.asarray(W2, np.float32)
    b1 = np.asarray(b1, np.float32)
    b2 = np.asarray(b2, np.float32)
    in_maps = []
    for e in range(NUM_EXPERTS):
        xe = np.zeros((C, DIM), np.float32)
        xe[: len(tok[e])] = xt[tok[e]]
        # [C, D] -> [nch, P, KS, csz] with d = ks*P + p, chunk-contiguous
        xt_full = xe.T.reshape(KS, P, C).transpose(1, 0, 2)
        xt_pack = np.ascontiguousarray(
            np.stack([xt_full[:, :, t0:t1] for (t0, t1) in chunks])
        ).astype(_BF16)
        # W1[d, h] -> [HC//2, P, KS, 2, P] : [hc2, p_d, ks, half, p_h]
        w1_pack = np.ascontiguousarray(
            W1[e].reshape(KS, P, HC // 2, 2, P).transpose(2, 1, 0, 3, 4)
        ).astype(_BF16)
        # W2[h, d] -> [DC, P, HC, P] : [dc, p_h, hs, p_d]
        w2_pack = np.ascontiguousarray(
            W2[e].reshape(HC, P, DC, P).transpose(2, 1, 0, 3)
        ).astype(_BF16)
        b1_pack = np.ascontiguousarray(b1[e].reshape(HC, P).T)
        b2_pack = np.ascontiguousarray(b2[e].reshape(DC, P).T)
        in_maps.append(
            {
                "xt": xt_pack,
                "w1": w1_pack,
                "b1": b1_pack,
                "w2": w2_pack,
                "b2": b2_pack,
            }
        )

    global _last_in_maps
    _last_in_maps = in_maps
    res = run_bass_kernel_spmd(nc, in_maps, core_ids=list(range(NUM_EXPERTS)))

    out = np.zeros((T, DIM), np.float32)
    for e in range(NUM_EXPERTS):
        yt = np.asarray(res.results[e]["yt"], np.float32)  # [DC, P, C]
        y = yt.reshape(DIM, C)[:, : len(tok[e])].T  # [n_e, D]
        out[tok[e]] += wgt[e][:, None] * y

    return out.reshape(x.shape), aux


# revision 12
# speedup vs baseline: 1.0104x; 1.0011x over previous
"""MoE (8 experts, top-2) Trainium2 kernel.

Strategy (expert-parallel, per the sharding hint):
  - Host computes the tiny router (logits/softmax/top-2/gates + aux loss).
  - Each of the 8 NeuronCores gets one expert: the tokens routed to that
    expert are gathered into a transposed [D, C] activation block (C =
    max expert load, padded), plus that expert's W1/b1/W2/b2.
  - On-core: H^T = gelu(W1^T @ X^T + b1), Y^T = W2^T @ H^T + b2, both as
    chains of 128x128xN matmuls that keep the contraction on the partition
    dim, so no on-device transposes are needed anywhere.
  - Host scatter-adds gate * Y back into the full output (exact: non-top-2
    experts have combine weight 0 in the reference).

Shapes are hardcoded for the graded problem: x [1,2048,1024], E=8, D=1024,
H=4096, top-2. The Bass program is built after routing is known, so the
capacity C adapts to the actual input.
"""

import numpy as np
import ml_dtypes

DIM = 1024
HIDDEN = 4096
NUM_EXPERTS = 8
TOP_K = 2
AUX_COEF = 0.01

P = 128
KS = DIM // P      # 8   k-subtiles for MM1 contraction
HC = HIDDEN // P   # 32  h-chunks (MM1 output partitions / MM2 contraction)
DC = DIM // P      # 8   d-chunks (MM2 output partitions)

_BF16 = ml_dtypes.bfloat16

_nc_cache = {}


def _build_bass(C, chunks):
    import concourse.tile as tile
    from concourse import bacc, mybir

    dt = mybir.dt
    act = mybir.ActivationFunctionType

    nc = bacc.Bacc(None, target_bir_lowering=False, debug=False)

    xt_d = nc.dram_tensor("xt", [len(chunks), P, KS, chunks[0][1]], dt.bfloat16, kind="ExternalInput")
    w1h_d = nc.dram_tensor("w1h", [4, P, KS, P], dt.bfloat16, kind="ExternalInput")
    w1_d = nc.dram_tensor("w1", [HC // 2, P, KS, 2, P], dt.bfloat16, kind="ExternalInput")
    b1_d = nc.dram_tensor("b1", [P, HC], dt.float32, kind="ExternalInput")
    w2_d = nc.dram_tensor("w2", [DC, P, HC, P], dt.bfloat16, kind="ExternalInput")
    b2_d = nc.dram_tensor("b2", [P, DC], dt.float32, kind="ExternalInput")
    yt_d = nc.dram_tensor("yt", [DC, P, C], dt.float32, kind="ExternalOutput")

    with tile.TileContext(nc) as tc:
        with (
            tc.tile_pool(name="big", bufs=1) as big,
            tc.tile_pool(name="w1p", bufs=4) as w1p,
            tc.tile_pool(name="w2p", bufs=4) as w2p,
            tc.tile_pool(name="yp", bufs=3) as yp,
            tc.tile_pool(name="ps", bufs=6, space="PSUM") as psp,
            tc.tile_pool(name="wps", bufs=1, space="PSUM") as wpsp,
        ):
            # PE warmup: dummy matmuls on a zeroed tile while input DMAs
            # are in flight, so the HAM clock-gate is already at 2.4 GHz
            # when the first real matmul issues (saves the ~3.4us cold
            # window at half rate).
            warm = big.tile([P, 256], dt.bfloat16)
            nc.any.memzero(warm[:])
            wps = wpsp.tile([P, 512], dt.float32)
            for _ in range(16):
                nc.tensor.matmul(wps[:, :256], warm[:, :P], warm[:])

            # xt + weights all on the sync HWDGE ring in exact consumption
            # order (queue is FIFO; packets of different queues round-robin,
            # so the critical path must be alone at the head of one queue).
            # The first real matmul needs only w1[hc=0] + xt chunk 0.
            xt_c = [big.tile([P, KS, t1 - t0], dt.bfloat16, tag=f"xt{ci}", name="xt")
                    for ci, (t0, t1) in enumerate(chunks)]
            w1_singles = [w1p.tile([P, KS, P], dt.bfloat16, tag="w1s", name="w1s")
                          for _ in range(4)]
            nc.sync.dma_start(w1_singles[0][:], w1h_d[0])
            for ci in range(len(chunks)):
                nc.sync.dma_start(xt_c[ci][:], xt_d[ci])
                if ci + 1 < 4:
                    nc.sync.dma_start(w1_singles[ci + 1][:], w1h_d[ci + 1])
            for hc in range(len(chunks) + 1, 4):
                nc.sync.dma_start(w1_singles[hc][:], w1h_d[hc])
            b1_s = big.tile([P, HC], dt.float32)
            nc.scalar.dma_start(b1_s[:], b1_d[:])
            b2_s = big.tile([P, DC], dt.float32)
            nc.scalar.dma_start(b2_s[:], b2_d[:])
            h_s = big.tile([P, HC, C], dt.bfloat16)

            # MM1: H^T[h, t] = gelu(sum_d W1[d, h] * X^T[d, t] + b1[h])
            def mm1(hc, w1_lhsT):
                for ci, (t0, t1) in enumerate(chunks):
                    n = t1 - t0
                    ps = psp.tile([P, 512], dt.float32, tag="ps", name="ps")[:, :n]
                    for ks in range(KS):
                        nc.tensor.matmul(
                            ps,
                            w1_lhsT(ks),
                            xt_c[ci][:, ks],
                            start=(ks == 0),
                            stop=(ks == KS - 1),
                        )
                    nc.scalar.activation(
                        h_s[:, hc, t0:t1], ps, act.Gelu, bias=b1_s[:, hc : hc + 1]
                    )

            for hc in range(4):
                mm1(hc, lambda ks, t=w1_singles[hc]: t[:, ks])
            for hc2 in range(2, HC // 2):
                w1_t = w1p.tile([P, KS, 2, P], dt.bfloat16)
                nc.sync.dma_start(w1_t[:], w1_d[hc2])
                for half in range(2):
                    mm1(2 * hc2 + half, lambda ks, t=w1_t, h=half: t[:, ks, h])

            # MM2: Y^T[d, t] = sum_h W2[h, d] * H^T[h, t] + b2[d]
            for dc in range(DC):
                w2_t = w2p.tile([P, HC, P], dt.bfloat16)
                nc.sync.dma_start(w2_t[:], w2_d[dc])
                y_t = yp.tile([P, C], dt.float32, tag="y", name="y")
                for (t0, t1) in chunks:
                    n = t1 - t0
                    ps = psp.tile([P, 512], dt.float32, tag="ps", name="ps")[:, :n]
                    for hs in range(HC):
                        nc.tensor.matmul(
                            ps,
                            w2_t[:, hs],
                            h_s[:, hs, t0:t1],
                            start=(hs == 0),
                            stop=(hs == HC - 1),
                        )
                    nc.scalar.activation(
                        y_t[:, t0:t1], ps, act.Identity, bias=b2_s[:, dc : dc + 1]
                    )
                nc.scalar.dma_start(yt_d[dc], y_t[:])

    nc.compile()
    return nc


def _route(x, router_w):
    """Replicates the reference router in fp32 numpy."""
    xt = x.reshape(-1, DIM).astype(np.float32)
    logits = xt @ router_w.T.astype(np.float32)
    m = logits.max(-1, keepdims=True)
    e = np.exp(logits - m)
    probs = e / e.sum(-1, keepdims=True)
    # top-2, ties -> lower index first (matches jax.lax.top_k)
    idx = np.argsort(-probs, axis=-1, kind="stable")[:, :TOP_K]
    gates = np.take_along_axis(probs, idx, axis=-1)
    gates = gates / (gates.sum(-1, keepdims=True) + 1e-9)
    # aux loss
    counts = np.zeros(NUM_EXPERTS, np.float64)
    for k in range(TOP_K):
        counts += np.bincount(idx[:, k], minlength=NUM_EXPERTS)
    onehot_mean = (counts / (idx.shape[0] * TOP_K)).astype(np.float32)
    aux = np.float32(AUX_COEF * NUM_EXPERTS * (onehot_mean * probs.mean(0)).sum())
    return xt, idx, gates, aux


def kernel(x, router_w, W1, b1, W2, b2):
    from concourse.bass_utils import run_bass_kernel_spmd

    xt, idx, gates, aux = _route(x, router_w)
    T = xt.shape[0]

    # per-expert token lists and combine weights
    tok, wgt = [], []
    for e in range(NUM_EXPERTS):
        sel0 = idx[:, 0] == e
        sel1 = idx[:, 1] == e
        t = np.flatnonzero(sel0 | sel1)
        w = np.where(sel0[t], gates[t, 0], gates[t, 1]).astype(np.float32)
        tok.append(t)
        wgt.append(w)

    maxload = max(len(t) for t in tok)
    n_chunks = max(1, -(-maxload // 512))
    csz = -(-(-(-maxload // n_chunks)) // 16) * 16  # chunk size, mult of 16
    C = csz * n_chunks
    chunks = tuple((i * csz, (i + 1) * csz) for i in range(n_chunks))

    key = (C, chunks)
    if key not in _nc_cache:
        _nc_cache[key] = _build_bass(C, chunks)
    nc = _nc_cache[key]

    # per-core input maps, weights pre-packed into the exact SBUF layouts
    W1 = np.asarray(W1, np.float32)
    W2 = np.asarray(W2, np.float32)
    b1 = np.asarray(b1, np.float32)
    b2 = np.asarray(b2, np.float32)
    in_maps = []
    for e in range(NUM_EXPERTS):
        xe = np.zeros((C, DIM), np.float32)
        xe[: len(tok[e])] = xt[tok[e]]
        # [C, D] -> [nch, P, KS, csz] with d = ks*P + p, chunk-contiguous
        xt_full = xe.T.reshape(KS, P, C).transpose(1, 0, 2)
        xt_pack = np.ascontiguousarray(
            np.stack([xt_full[:, :, t0:t1] for (t0, t1) in chunks])
        ).astype(_BF16)
        # W1[d, h] -> [HC//2, P, KS, 2, P] : [hc2, p_d, ks, half, p_h]
        w1_pack = np.ascontiguousarray(
            W1[e].reshape(KS, P, HC // 2, 2, P).transpose(2, 1, 0, 3, 4)
        ).astype(_BF16)
        # first 4 h-chunks also packed contiguously for the startup path
        w1h_pack = np.ascontiguousarray(
            W1[e][:, : 4 * P].reshape(KS, P, 4, P).transpose(2, 1, 0, 3)
        ).astype(_BF16)
        # W2[h, d] -> [DC, P, HC, P] : [dc, p_h, hs, p_d]
        w2_pack = np.ascontiguousarray(
            W2[e].reshape(HC, P, DC, P).transpose(2, 1, 0, 3)
        ).astype(_BF16)
        b1_pack = np.ascontiguousarray(b1[e].reshape(HC, P).T)
        b2_pack = np.ascontiguousarray(b2[e].reshape(DC, P).T)
        in_maps.append(
            {
                "xt": xt_pack,
                "w1": w1_pack,
                "w1h": w1h_pack,
                "b1": b1_pack,
                "w2": w2_pack,
                "b2": b2_pack,
            }
        )

    global _last_in_maps
    _last_in_maps = in_maps
    res = run_bass_kernel_spmd(nc, in_maps, core_ids=list(range(NUM_EXPERTS)))

    out = np.zeros((T, DIM), np.float32)
    for e in range(NUM_EXPERTS):
        yt = np.asarray(res.results[e]["yt"], np.float32)  # [DC, P, C]
        y = yt.reshape(DIM, C)[:, : len(tok[e])].T  # [n_e, D]
        out[tok[e]] += wgt[e][:, None] * y

    return out.reshape(x.shape), aux


# revision 13
# speedup vs baseline: 1.0325x; 1.0218x over previous
"""MoE (8 experts, top-2) Trainium2 kernel.

Strategy (expert-parallel, per the sharding hint):
  - Host computes the tiny router (logits/softmax/top-2/gates + aux loss).
  - Each of the 8 NeuronCores gets one expert: the tokens routed to that
    expert are gathered into a transposed [D, C] activation block (C =
    max expert load, padded), plus that expert's W1/b1/W2/b2.
  - On-core: H^T = gelu(W1^T @ X^T + b1), Y^T = W2^T @ H^T + b2, both as
    chains of 128x128xN matmuls that keep the contraction on the partition
    dim, so no on-device transposes are needed anywhere.
  - Host scatter-adds gate * Y back into the full output (exact: non-top-2
    experts have combine weight 0 in the reference).

Shapes are hardcoded for the graded problem: x [1,2048,1024], E=8, D=1024,
H=4096, top-2. The Bass program is built after routing is known, so the
capacity C adapts to the actual input.
"""

import numpy as np
import ml_dtypes

DIM = 1024
HIDDEN = 4096
NUM_EXPERTS = 8
TOP_K = 2
AUX_COEF = 0.01

P = 128
KS = DIM // P      # 8   k-subtiles for MM1 contraction
HC = HIDDEN // P   # 32  h-chunks (MM1 output partitions / MM2 contraction)
DC = DIM // P      # 8   d-chunks (MM2 output partitions)

_BF16 = ml_dtypes.bfloat16

_nc_cache = {}


def _build_bass(C, chunks):
    import concourse.tile as tile
    from concourse import bacc, mybir

    dt = mybir.dt
    act = mybir.ActivationFunctionType

    nc = bacc.Bacc(None, target_bir_lowering=False, debug=False)

    xt_d = nc.dram_tensor("xt", [len(chunks), P, KS, chunks[0][1]], dt.bfloat16, kind="ExternalInput")
    w1h_d = nc.dram_tensor("w1h", [4, P, KS, P], dt.bfloat16, kind="ExternalInput")
    w1_d = nc.dram_tensor("w1", [HC // 2, P, KS, 2, P], dt.bfloat16, kind="ExternalInput")
    b1_d = nc.dram_tensor("b1", [P, HC], dt.float32, kind="ExternalInput")
    w2_d = nc.dram_tensor("w2", [DC, P, HC, P], dt.bfloat16, kind="ExternalInput")
    b2_d = nc.dram_tensor("b2", [P, DC], dt.float32, kind="ExternalInput")
    yt_d = nc.dram_tensor("yt", [DC, P, C], dt.float32, kind="ExternalOutput")

    with tile.TileContext(nc) as tc:
        with (
            tc.tile_pool(name="big", bufs=1) as big,
            tc.tile_pool(name="w1p", bufs=4) as w1p,
            tc.tile_pool(name="w2p", bufs=4) as w2p,
            tc.tile_pool(name="yp", bufs=3) as yp,
            tc.tile_pool(name="ps", bufs=6, space="PSUM") as psp,
            tc.tile_pool(name="wps", bufs=1, space="PSUM") as wpsp,
        ):
            # PE warmup: dummy matmuls on a zeroed tile while input DMAs
            # are in flight, so the HAM clock-gate is already at 2.4 GHz
            # when the first real matmul issues (saves the ~3.4us cold
            # window at half rate).
            warm = big.tile([P, 256], dt.bfloat16)
            nc.any.memzero(warm[:])
            wps = wpsp.tile([P, 512], dt.float32)
            for _ in range(24):
                nc.tensor.matmul(wps[:, :256], warm[:, :P], warm[:])

            # xt + weights all on the sync HWDGE ring in exact consumption
            # order (queue is FIFO; packets of different queues round-robin,
            # so the critical path must be alone at the head of one queue).
            # The first real matmul needs only w1[hc=0] + xt chunk 0.
            xt_c = [big.tile([P, KS, t1 - t0], dt.bfloat16, tag=f"xt{ci}", name="xt")
                    for ci, (t0, t1) in enumerate(chunks)]
            w1_singles = [w1p.tile([P, KS, P], dt.bfloat16, tag="w1s", name="w1s")
                          for _ in range(4)]
            nc.sync.dma_start(w1_singles[0][:], w1h_d[0])
            for ci in range(len(chunks)):
                nc.sync.dma_start(xt_c[ci][:], xt_d[ci])
                if ci + 1 < 4:
                    nc.sync.dma_start(w1_singles[ci + 1][:], w1h_d[ci + 1])
            for hc in range(len(chunks) + 1, 4):
                nc.sync.dma_start(w1_singles[hc][:], w1h_d[hc])
            b1_s = big.tile([P, HC], dt.float32)
            nc.scalar.dma_start(b1_s[:], b1_d[:])
            b2_s = big.tile([P, DC], dt.float32)
            nc.scalar.dma_start(b2_s[:], b2_d[:])
            h_s = big.tile([P, HC, C], dt.bfloat16)

            # MM1: H^T[h, t] = gelu(sum_d W1[d, h] * X^T[d, t] + b1[h])
            def mm1(hc, w1_lhsT):
                for ci, (t0, t1) in enumerate(chunks):
                    n = t1 - t0
                    ps = psp.tile([P, 512], dt.float32, tag="ps", name="ps")[:, :n]
                    for ks in range(KS):
                        nc.tensor.matmul(
                            ps,
                            w1_lhsT(ks),
                            xt_c[ci][:, ks],
                            start=(ks == 0),
                            stop=(ks == KS - 1),
                        )
                    nc.scalar.activation(
                        h_s[:, hc, t0:t1], ps, act.Gelu, bias=b1_s[:, hc : hc + 1]
                    )

            for hc in range(4):
                mm1(hc, lambda ks, t=w1_singles[hc]: t[:, ks])
            for hc2 in range(2, HC // 2):
                w1_t = w1p.tile([P, KS, 2, P], dt.bfloat16)
                nc.sync.dma_start(w1_t[:], w1_d[hc2])
                for half in range(2):
                    mm1(2 * hc2 + half, lambda ks, t=w1_t, h=half: t[:, ks, h])

            # MM2: Y^T[d, t] = sum_h W2[h, d] * H^T[h, t] + b2[d]
            for dc in range(DC):
                w2_t = w2p.tile([P, HC, P], dt.bfloat16)
                nc.sync.dma_start(w2_t[:], w2_d[dc])
                y_t = yp.tile([P, C], dt.float32, tag="y", name="y")
                for (t0, t1) in chunks:
                    n = t1 - t0
                    ps = psp.tile([P, 512], dt.float32, tag="ps", name="ps")[:, :n]
                    for hs in range(HC):
                        nc.tensor.matmul(
                            ps,
                            w2_t[:, hs],
                            h_s[:, hs, t0:t1],
                            start=(hs == 0),
                            stop=(hs == HC - 1),
                        )
                    nc.scalar.activation(
                        y_t[:, t0:t1], ps, act.Identity, bias=b2_s[:, dc : dc + 1]
                    )
                nc.scalar.dma_start(yt_d[dc], y_t[:])

    nc.compile()
    return nc


def _route(x, router_w):
    """Replicates the reference router in fp32 numpy."""
    xt = x.reshape(-1, DIM).astype(np.float32)
    logits = xt @ router_w.T.astype(np.float32)
    m = logits.max(-1, keepdims=True)
    e = np.exp(logits - m)
    probs = e / e.sum(-1, keepdims=True)
    # top-2, ties -> lower index first (matches jax.lax.top_k)
    idx = np.argsort(-probs, axis=-1, kind="stable")[:, :TOP_K]
    gates = np.take_along_axis(probs, idx, axis=-1)
    gates = gates / (gates.sum(-1, keepdims=True) + 1e-9)
    # aux loss
    counts = np.zeros(NUM_EXPERTS, np.float64)
    for k in range(TOP_K):
        counts += np.bincount(idx[:, k], minlength=NUM_EXPERTS)
    onehot_mean = (counts / (idx.shape[0] * TOP_K)).astype(np.float32)
    aux = np.float32(AUX_COEF * NUM_EXPERTS * (onehot_mean * probs.mean(0)).sum())
    return xt, idx, gates, aux


def kernel(x, router_w, W1, b1, W2, b2):
    from concourse.bass_utils import run_bass_kernel_spmd

    xt, idx, gates, aux = _route(x, router_w)
    T = xt.shape[0]

    # per-expert token lists and combine weights
    tok, wgt = [], []
    for e in range(NUM_EXPERTS):
        sel0 = idx[:, 0] == e
        sel1 = idx[:, 1] == e
        t = np.flatnonzero(sel0 | sel1)
        w = np.where(sel0[t], gates[t, 0], gates[t, 1]).astype(np.float32)
        tok.append(t)
        wgt.append(w)

    maxload = max(len(t) for t in tok)
    n_chunks = max(1, -(-maxload // 512))
    csz = -(-(-(-maxload // n_chunks)) // 16) * 16  # chunk size, mult of 16
    C = csz * n_chunks
    chunks = tuple((i * csz, (i + 1) * csz) for i in range(n_chunks))

    key = (C, chunks)
    if key not in _nc_cache:
        _nc_cache[key] = _build_bass(C, chunks)
    nc = _nc_cache[key]

    # per-core input maps, weights pre-packed into the exact SBUF layouts
    W1 = np.asarray(W1, np.float32)
    W2 = np.asarray(W2, np.float32)
    b1 = np.asarray(b1, np.float32)
    b2 = np.asarray(b2, np.float32)
    in_maps = []
    for e in range(NUM_EXPERTS):
        xe = np.zeros((C, DIM), np.float32)
        xe[: len(tok[e])] = xt[tok[e]]
        # [C, D] -> [nch, P, KS, csz] with d = ks*P + p, chunk-contiguous
        xt_full = xe.T.reshape(KS, P, C).transpose(1, 0, 2)
        xt_pack = np.ascontiguousarray(
            np.stack([xt_full[:, :, t0:t1] for (t0, t1) in chunks])
        ).astype(_BF16)
        # W1[d, h] -> [HC//2, P, KS, 2, P] : [hc2, p_d, ks, half, p_h]
        w1_pack = np.ascontiguousarray(
            W1[e].reshape(KS, P, HC // 2, 2, P).transpose(2, 1, 0, 3, 4)
        ).astype(_BF16)
        # first 4 h-chunks also packed contiguously for the startup path
        w1h_pack = np.ascontiguousarray(
            W1[e][:, : 4 * P].reshape(KS, P, 4, P).transpose(2, 1, 0, 3)
        ).astype(_BF16)
        # W2[h, d] -> [DC, P, HC, P] : [dc, p_h, hs, p_d]
        w2_pack = np.ascontiguousarray(
            W2[e].reshape(HC, P, DC, P).transpose(2, 1, 0, 3)
        ).astype(_BF16)
        b1_pack = np.ascontiguousarray(b1[e].reshape(HC, P).T)
        b2_pack = np.ascontiguousarray(b2[e].reshape(DC, P).T)
        in_maps.append(
            {
                "xt": xt_pack,
                "w1": w1_pack,
                "w1h": w1h_pack,
                "b1": b1_pack,
                "w2": w2_pack,
                "b2": b2_pack,
            }
        )

    global _last_in_maps
    _last_in_maps = in_maps
    res = run_bass_kernel_spmd(nc, in_maps, core_ids=list(range(NUM_EXPERTS)))

    out = np.zeros((T, DIM), np.float32)
    for e in range(NUM_EXPERTS):
        yt = np.asarray(res.results[e]["yt"], np.float32)  # [DC, P, C]
        y = yt.reshape(DIM, C)[:, : len(tok[e])].T  # [n_e, D]
        out[tok[e]] += wgt[e][:, None] * y

    return out.reshape(x.shape), aux


# revision 14
# speedup vs baseline: 1.0358x; 1.0032x over previous
"""MoE (8 experts, top-2) Trainium2 kernel.

Strategy (expert-parallel, per the sharding hint):
  - Host computes the tiny router (logits/softmax/top-2/gates + aux loss).
  - Each of the 8 NeuronCores gets one expert: the tokens routed to that
    expert are gathered into a transposed [D, C] activation block (C =
    max expert load, padded), plus that expert's W1/b1/W2/b2.
  - On-core: H^T = gelu(W1^T @ X^T + b1), Y^T = W2^T @ H^T + b2, both as
    chains of 128x128xN matmuls that keep the contraction on the partition
    dim, so no on-device transposes are needed anywhere.
  - Host scatter-adds gate * Y back into the full output (exact: non-top-2
    experts have combine weight 0 in the reference).

Shapes are hardcoded for the graded problem: x [1,2048,1024], E=8, D=1024,
H=4096, top-2. The Bass program is built after routing is known, so the
capacity C adapts to the actual input.
"""

import numpy as np
import ml_dtypes

DIM = 1024
HIDDEN = 4096
NUM_EXPERTS = 8
TOP_K = 2
AUX_COEF = 0.01

P = 128
KS = DIM // P      # 8   k-subtiles for MM1 contraction
HC = HIDDEN // P   # 32  h-chunks (MM1 output partitions / MM2 contraction)
DC = DIM // P      # 8   d-chunks (MM2 output partitions)

_BF16 = ml_dtypes.bfloat16

_nc_cache = {}


def _build_bass(C, chunks):
    import concourse.tile as tile
    from concourse import bacc, mybir

    dt = mybir.dt
    act = mybir.ActivationFunctionType

    nc = bacc.Bacc(None, target_bir_lowering=False, debug=False)

    xt_d = nc.dram_tensor("xt", [len(chunks), P, KS, chunks[0][1]], dt.bfloat16, kind="ExternalInput")
    w1h_d = nc.dram_tensor("w1h", [4, P, KS, P], dt.bfloat16, kind="ExternalInput")
    w1_d = nc.dram_tensor("w1", [HC // 2, P, KS, 2, P], dt.bfloat16, kind="ExternalInput")
    b1_d = nc.dram_tensor("b1", [P, HC], dt.float32, kind="ExternalInput")
    w2_d = nc.dram_tensor("w2", [DC, P, HC, P], dt.bfloat16, kind="ExternalInput")
    b2_d = nc.dram_tensor("b2", [P, DC], dt.float32, kind="ExternalInput")
    yt_d = nc.dram_tensor("yt", [DC, P, C], dt.float32, kind="ExternalOutput")

    with tile.TileContext(nc) as tc:
        with (
            tc.tile_pool(name="big", bufs=1) as big,
            tc.tile_pool(name="w1p", bufs=4) as w1p,
            tc.tile_pool(name="w2p", bufs=4) as w2p,
            tc.tile_pool(name="yp", bufs=3) as yp,
            tc.tile_pool(name="ps", bufs=6, space="PSUM") as psp,
            tc.tile_pool(name="wps", bufs=1, space="PSUM") as wpsp,
        ):
            # PE warmup: dummy matmuls on a zeroed tile while input DMAs
            # are in flight, so the HAM clock-gate is already at 2.4 GHz
            # when the first real matmul issues (saves the ~3.4us cold
            # window at half rate).
            warm = big.tile([P, 256], dt.bfloat16)
            nc.any.memzero(warm[:])
            wps = wpsp.tile([P, 512], dt.float32)
            for _ in range(24):
                nc.tensor.matmul(wps[:, :256], warm[:, :P], warm[:])

            # xt + weights all on the sync HWDGE ring in exact consumption
            # order (queue is FIFO; packets of different queues round-robin,
            # so the critical path must be alone at the head of one queue).
            # The first real matmul needs only w1[hc=0] + xt chunk 0.
            xt_c = [big.tile([P, KS, t1 - t0], dt.bfloat16, tag=f"xt{ci}", name="xt")
                    for ci, (t0, t1) in enumerate(chunks)]
            w1_singles = [w1p.tile([P, KS, P], dt.bfloat16, tag="w1s", name="w1s")
                          for _ in range(4)]
            nc.sync.dma_start(w1_singles[0][:], w1h_d[0])
            for ci in range(len(chunks)):
                nc.sync.dma_start(xt_c[ci][:], xt_d[ci])
                if ci + 1 < 4:
                    nc.sync.dma_start(w1_singles[ci + 1][:], w1h_d[ci + 1])
            for hc in range(len(chunks) + 1, 4):
                nc.sync.dma_start(w1_singles[hc][:], w1h_d[hc])
            b1_s = big.tile([P, HC], dt.float32)
            nc.scalar.dma_start(b1_s[:], b1_d[:])
            b2_s = big.tile([P, DC], dt.float32)
            nc.scalar.dma_start(b2_s[:], b2_d[:])
            h_s = big.tile([P, HC, C], dt.bfloat16)

            # MM1: H^T[h, t] = gelu(sum_d W1[d, h] * X^T[d, t] + b1[h])
            def mm1_group(hc, ci, w1_lhsT):
                t0, t1 = chunks[ci]
                n = t1 - t0
                ps = psp.tile([P, 512], dt.float32, tag="ps", name="ps")[:, :n]
                for ks in range(KS):
                    nc.tensor.matmul(
                        ps,
                        w1_lhsT(ks),
                        xt_c[ci][:, ks],
                        start=(ks == 0),
                        stop=(ks == KS - 1),
                    )
                nc.scalar.activation(
                    h_s[:, hc, t0:t1], ps, act.Gelu, bias=b1_s[:, hc : hc + 1]
                )

            def mm1(hc, w1_lhsT):
                for ci in range(len(chunks)):
                    mm1_group(hc, ci, w1_lhsT)

            # first 4 h-chunks run chunk-0 first so the xt chunk-1 DMA has
            # ~4 extra us to land before anything consumes it
            for ci in range(len(chunks)):
                for hc in range(4):
                    mm1_group(hc, ci, lambda ks, t=w1_singles[hc]: t[:, ks])
            for hc2 in range(2, HC // 2):
                w1_t = w1p.tile([P, KS, 2, P], dt.bfloat16)
                nc.sync.dma_start(w1_t[:], w1_d[hc2])
                for half in range(2):
                    mm1(2 * hc2 + half, lambda ks, t=w1_t, h=half: t[:, ks, h])

            # MM2: Y^T[d, t] = sum_h W2[h, d] * H^T[h, t] + b2[d]
            for dc in range(DC):
                w2_t = w2p.tile([P, HC, P], dt.bfloat16)
                nc.sync.dma_start(w2_t[:], w2_d[dc])
                y_t = yp.tile([P, C], dt.float32, tag="y", name="y")
                for (t0, t1) in chunks:
                    n = t1 - t0
                    ps = psp.tile([P, 512], dt.float32, tag="ps", name="ps")[:, :n]
                    for hs in range(HC):
                        nc.tensor.matmul(
                            ps,
                            w2_t[:, hs],
                            h_s[:, hs, t0:t1],
                            start=(hs == 0),
                            stop=(hs == HC - 1),
                        )
                    nc.scalar.activation(
                        y_t[:, t0:t1], ps, act.Identity, bias=b2_s[:, dc : dc + 1]
                    )
                nc.scalar.dma_start(yt_d[dc], y_t[:])

    nc.compile()
    return nc


def _route(x, router_w):
    """Replicates the reference router in fp32 numpy."""
    xt = x.reshape(-1, DIM).astype(np.float32)
    logits = xt @ router_w.T.astype(np.float32)
    m = logits.max(-1, keepdims=True)
    e = np.exp(logits - m)
    probs = e / e.sum(-1, keepdims=True)
    # top-2, ties -> lower index first (matches jax.lax.top_k)
    idx = np.argsort(-probs, axis=-1, kind="stable")[:, :TOP_K]
    gates = np.take_along_axis(probs, idx, axis=-1)
    gates = gates / (gates.sum(-1, keepdims=True) + 1e-9)
    # aux loss
    counts = np.zeros(NUM_EXPERTS, np.float64)
    for k in range(TOP_K):
        counts += np.bincount(idx[:, k], minlength=NUM_EXPERTS)
    onehot_mean = (counts / (idx.shape[0] * TOP_K)).astype(np.float32)
    aux = np.float32(AUX_COEF * NUM_EXPERTS * (onehot_mean * probs.mean(0)).sum())
    return xt, idx, gates, aux


def kernel(x, router_w, W1, b1, W2, b2):
    from concourse.bass_utils import run_bass_kernel_spmd

    xt, idx, gates, aux = _route(x, router_w)
    T = xt.shape[0]

    # per-expert token lists and combine weights
    tok, wgt = [], []
    for e in range(NUM_EXPERTS):
        sel0 = idx[:, 0] == e
        sel1 = idx[:, 1] == e
        t = np.flatnonzero(sel0 | sel1)
        w = np.where(sel0[t], gates[t, 0], gates[t, 1]).astype(np.float32)
        tok.append(t)
        wgt.append(w)

    maxload = max(len(t) for t in tok)
    n_chunks = max(1, -(-maxload // 512))
    csz = -(-(-(-maxload // n_chunks)) // 16) * 16  # chunk size, mult of 16
    C = csz * n_chunks
    chunks = tuple((i * csz, (i + 1) * csz) for i in range(n_chunks))

    key = (C, chunks)
    if key not in _nc_cache:
        _nc_cache[key] = _build_bass(C, chunks)
    nc = _nc_cache[key]

    # per-core input maps, weights pre-packed into the exact SBUF layouts
    W1 = np.asarray(W1, np.float32)
    W2 = np.asarray(W2, np.float32)
    b1 = np.asarray(b1, np.float32)
    b2 = np.asarray(b2, np.float32)
    in_maps = []
    for e in range(NUM_EXPERTS):
        xe = np.zeros((C, DIM), np.float32)
        xe[: len(tok[e])] = xt[tok[e]]
        # [C, D] -> [nch, P, KS, csz] with d = ks*P + p, chunk-contiguous
        xt_full = xe.T.reshape(KS, P, C).transpose(1, 0, 2)
        xt_pack = np.ascontiguousarray(
            np.stack([xt_full[:, :, t0:t1] for (t0, t1) in chunks])
        ).astype(_BF16)
        # W1[d, h] -> [HC//2, P, KS, 2, P] : [hc2, p_d, ks, half, p_h]
        w1_pack = np.ascontiguousarray(
            W1[e].reshape(KS, P, HC // 2, 2, P).transpose(2, 1, 0, 3, 4)
        ).astype(_BF16)
        # first 4 h-chunks also packed contiguously for the startup path
        w1h_pack = np.ascontiguousarray(
            W1[e][:, : 4 * P].reshape(KS, P, 4, P).transpose(2, 1, 0, 3)
        ).astype(_BF16)
        # W2[h, d] -> [DC, P, HC, P] : [dc, p_h, hs, p_d]
        w2_pack = np.ascontiguousarray(
            W2[e].reshape(HC, P, DC, P).transpose(2, 1, 0, 3)
        ).astype(_BF16)
        b1_pack = np.ascontiguousarray(b1[e].reshape(HC, P).T)
        b2_pack = np.ascontiguousarray(b2[e].reshape(DC, P).T)
        in_maps.append(
            {
                "xt": xt_pack,
                "w1": w1_pack,
                "w1h": w1h_pack,
                "b1": b1_pack,
                "w2": w2_pack,
                "b2": b2_pack,
            }
        )

    global _last_in_maps
    _last_in_maps = in_maps
    res = run_bass_kernel_spmd(nc, in_maps, core_ids=list(range(NUM_EXPERTS)))

    out = np.zeros((T, DIM), np.float32)
    for e in range(NUM_EXPERTS):
        yt = np.asarray(res.results[e]["yt"], np.float32)  # [DC, P, C]
        y = yt.reshape(DIM, C)[:, : len(tok[e])].T  # [n_e, D]
        out[tok[e]] += wgt[e][:, None] * y

    return out.reshape(x.shape), aux


# revision 15
# speedup vs baseline: 1.0507x; 1.0144x over previous
"""MoE (8 experts, top-2) Trainium2 kernel.

Strategy (expert-parallel, per the sharding hint):
  - Host computes the tiny router (logits/softmax/top-2/gates + aux loss).
  - Each of the 8 NeuronCores gets one expert: the tokens routed to that
    expert are gathered into a transposed [D, C] activation block (C =
    max expert load, padded), plus that expert's W1/b1/W2/b2.
  - On-core: H^T = gelu(W1^T @ X^T + b1), Y^T = W2^T @ H^T + b2, both as
    chains of 128x128xN matmuls that keep the contraction on the partition
    dim, so no on-device transposes are needed anywhere.
  - Host scatter-adds gate * Y back into the full output (exact: non-top-2
    experts have combine weight 0 in the reference).

Shapes are hardcoded for the graded problem: x [1,2048,1024], E=8, D=1024,
H=4096, top-2. The Bass program is built after routing is known, so the
capacity C adapts to the actual input.
"""

import numpy as np
import ml_dtypes

DIM = 1024
HIDDEN = 4096
NUM_EXPERTS = 8
TOP_K = 2
AUX_COEF = 0.01

P = 128
KS = DIM // P      # 8   k-subtiles for MM1 contraction
HC = HIDDEN // P   # 32  h-chunks (MM1 output partitions / MM2 contraction)
DC = DIM // P      # 8   d-chunks (MM2 output partitions)

_BF16 = ml_dtypes.bfloat16

_nc_cache = {}


def _build_bass(C, chunks):
    import concourse.tile as tile
    from concourse import bacc, mybir

    dt = mybir.dt
    act = mybir.ActivationFunctionType

    nc = bacc.Bacc(None, target_bir_lowering=False, debug=False)

    xt_d = nc.dram_tensor("xt", [len(chunks), P, KS, chunks[0][1]], dt.bfloat16, kind="ExternalInput")
    w1h_d = nc.dram_tensor("w1h", [4, P, KS, P], dt.bfloat16, kind="ExternalInput")
    w1_d = nc.dram_tensor("w1", [HC // 2, P, KS, 2, P], dt.bfloat16, kind="ExternalInput")
    b1_d = nc.dram_tensor("b1", [P, HC], dt.float32, kind="ExternalInput")
    w2_d = nc.dram_tensor("w2", [DC, P, HC, P], dt.bfloat16, kind="ExternalInput")
    b2_d = nc.dram_tensor("b2", [P, DC], dt.float32, kind="ExternalInput")
    yt_d = nc.dram_tensor("yt", [DC, P, C], dt.float32, kind="ExternalOutput")

    with tile.TileContext(nc) as tc:
        with (
            tc.tile_pool(name="big", bufs=1) as big,
            tc.tile_pool(name="w1p", bufs=4) as w1p,
            tc.tile_pool(name="w2p", bufs=4) as w2p,
            tc.tile_pool(name="yp", bufs=3) as yp,
            tc.tile_pool(name="ps", bufs=6, space="PSUM") as psp,
            tc.tile_pool(name="wps", bufs=1, space="PSUM") as wpsp,
        ):
            # PE warmup: dummy matmuls on a zeroed tile while input DMAs
            # are in flight, so the HAM clock-gate is already at 2.4 GHz
            # when the first real matmul issues (saves the ~3.4us cold
            # window at half rate).
            warm = big.tile([P, 256], dt.bfloat16)
            nc.any.memzero(warm[:])
            wps = wpsp.tile([P, 512], dt.float32)
            for _ in range(24):
                nc.tensor.matmul(wps[:, :256], warm[:, :P], warm[:])

            # xt + weights all on the sync HWDGE ring in exact consumption
            # order (queue is FIFO; packets of different queues round-robin,
            # so the critical path must be alone at the head of one queue).
            # The first real matmul needs only w1[hc=0] + xt chunk 0.
            xt_c = [big.tile([P, KS, t1 - t0], dt.bfloat16, tag=f"xt{ci}", name="xt")
                    for ci, (t0, t1) in enumerate(chunks)]
            w1_singles = [w1p.tile([P, KS, P], dt.bfloat16, tag="w1s", name="w1s")
                          for _ in range(4)]
            nc.sync.dma_start(w1_singles[0][:], w1h_d[0])
            nc.sync.dma_start(xt_c[0][:], xt_d[0])
            for hc in range(1, 4):
                nc.sync.dma_start(w1_singles[hc][:], w1h_d[hc])
            for ci in range(1, len(chunks)):
                nc.sync.dma_start(xt_c[ci][:], xt_d[ci])
            b1_s = big.tile([P, HC], dt.float32)
            nc.scalar.dma_start(b1_s[:], b1_d[:])
            b2_s = big.tile([P, DC], dt.float32)
            nc.scalar.dma_start(b2_s[:], b2_d[:])
            h_s = big.tile([P, HC, C], dt.bfloat16)

            # MM1: H^T[h, t] = gelu(sum_d W1[d, h] * X^T[d, t] + b1[h])
            def mm1_group(hc, ci, w1_lhsT):
                t0, t1 = chunks[ci]
                n = t1 - t0
                ps = psp.tile([P, 512], dt.float32, tag="ps", name="ps")[:, :n]
                for ks in range(KS):
                    nc.tensor.matmul(
                        ps,
                        w1_lhsT(ks),
                        xt_c[ci][:, ks],
                        start=(ks == 0),
                        stop=(ks == KS - 1),
                    )
                nc.scalar.activation(
                    h_s[:, hc, t0:t1], ps, act.Gelu, bias=b1_s[:, hc : hc + 1]
                )

            def mm1(hc, w1_lhsT):
                for ci in range(len(chunks)):
                    mm1_group(hc, ci, w1_lhsT)

            # first 4 h-chunks run chunk-0 first so the xt chunk-1 DMA has
            # ~4 extra us to land before anything consumes it
            for ci in range(len(chunks)):
                for hc in range(4):
                    mm1_group(hc, ci, lambda ks, t=w1_singles[hc]: t[:, ks])
            for hc2 in range(2, HC // 2):
                w1_t = w1p.tile([P, KS, 2, P], dt.bfloat16)
                nc.sync.dma_start(w1_t[:], w1_d[hc2])
                for half in range(2):
                    mm1(2 * hc2 + half, lambda ks, t=w1_t, h=half: t[:, ks, h])

            # MM2: Y^T[d, t] = sum_h W2[h, d] * H^T[h, t] + b2[d]
            for dc in range(DC):
                w2_t = w2p.tile([P, HC, P], dt.bfloat16)
                nc.sync.dma_start(w2_t[:], w2_d[dc])
                y_t = yp.tile([P, C], dt.float32, tag="y", name="y")
                for (t0, t1) in chunks:
                    n = t1 - t0
                    ps = psp.tile([P, 512], dt.float32, tag="ps", name="ps")[:, :n]
                    for hs in range(HC):
                        nc.tensor.matmul(
                            ps,
                            w2_t[:, hs],
                            h_s[:, hs, t0:t1],
                            start=(hs == 0),
                            stop=(hs == HC - 1),
                        )
                    nc.scalar.activation(
                        y_t[:, t0:t1], ps, act.Identity, bias=b2_s[:, dc : dc + 1]
                    )
                    nc.scalar.dma_start(yt_d[dc][:, t0:t1], y_t[:, t0:t1])

    nc.compile()
    return nc


def _route(x, router_w):
    """Replicates the reference router in fp32 numpy."""
    xt = x.reshape(-1, DIM).astype(np.float32)
    logits = xt @ router_w.T.astype(np.float32)
    m = logits.max(-1, keepdims=True)
    e = np.exp(logits - m)
    probs = e / e.sum(-1, keepdims=True)
    # top-2, ties -> lower index first (matches jax.lax.top_k)
    idx = np.argsort(-probs, axis=-1, kind="stable")[:, :TOP_K]
    gates = np.take_along_axis(probs, idx, axis=-1)
    gates = gates / (gates.sum(-1, keepdims=True) + 1e-9)
    # aux loss
    counts = np.zeros(NUM_EXPERTS, np.float64)
    for k in range(TOP_K):
        counts += np.bincount(idx[:, k], minlength=NUM_EXPERTS)
    onehot_mean = (counts / (idx.shape[0] * TOP_K)).astype(np.float32)
    aux = np.float32(AUX_COEF * NUM_EXPERTS * (onehot_mean * probs.mean(0)).sum())
    return xt, idx, gates, aux


def kernel(x, router_w, W1, b1, W2, b2):
    from concourse.bass_utils import run_bass_kernel_spmd

    xt, idx, gates, aux = _route(x, router_w)
    T = xt.shape[0]

    # per-expert token lists and combine weights
    tok, wgt = [], []
    for e in range(NUM_EXPERTS):
        sel0 = idx[:, 0] == e
        sel1 = idx[:, 1] == e
        t = np.flatnonzero(sel0 | sel1)
        w = np.where(sel0[t], gates[t, 0], gates[t, 1]).astype(np.float32)
        tok.append(t)
        wgt.append(w)

    maxload = max(len(t) for t in tok)
    n_chunks = max(1, -(-maxload // 512))
    csz = -(-(-(-maxload // n_chunks)) // 16) * 16  # chunk size, mult of 16
    C = csz * n_chunks
    chunks = tuple((i * csz, (i + 1) * csz) for i in range(n_chunks))

    key = (C, chunks)
    if key not in _nc_cache:
        _nc_cache[key] = _build_bass(C, chunks)
    nc = _nc_cache[key]

    # per-core input maps, weights pre-packed into the exact SBUF layouts
    W1 = np.asarray(W1, np.float32)
    W2 = np.asarray(W2, np.float32)
    b1 = np.asarray(b1, np.float32)
    b2 = np.asarray(b2, np.float32)
    in_maps = []
    for e in range(NUM_EXPERTS):
        xe = np.zeros((C, DIM), np.float32)
        xe[: len(tok[e])] = xt[tok[e]]
        # [C, D] -> [nch, P, KS, csz] with d = ks*P + p, chunk-contiguous
        xt_full = xe.T.reshape(KS, P, C).transpose(1, 0, 2)
        xt_pack = np.ascontiguousarray(
            np.stack([xt_full[:, :, t0:t1] for (t0, t1) in chunks])
        ).astype(_BF16)
        # W1[d, h] -> [HC//2, P, KS, 2, P] : [hc2, p_d, ks, half, p_h]
        w1_pack = np.ascontiguousarray(
            W1[e].reshape(KS, P, HC // 2, 2, P).transpose(2, 1, 0, 3, 4)
        ).astype(_BF16)
        # first 4 h-chunks also packed contiguously for the startup path
        w1h_pack = np.ascontiguousarray(
            W1[e][:, : 4 * P].reshape(KS, P, 4, P).transpose(2, 1, 0, 3)
        ).astype(_BF16)
        # W2[h, d] -> [DC, P, HC, P] : [dc, p_h, hs, p_d]
        w2_pack = np.ascontiguousarray(
            W2[e].reshape(HC, P, DC, P).transpose(2, 1, 0, 3)
        ).astype(_BF16)
        b1_pack = np.ascontiguousarray(b1[e].reshape(HC, P).T)
        b2_pack = np.ascontiguousarray(b2[e].reshape(DC, P).T)
        in_maps.append(
            {
                "xt": xt_pack,
                "w1": w1_pack,
                "w1h": w1h_pack,
                "b1": b1_pack,
                "w2": w2_pack,
                "b2": b2_pack,
            }
        )

    global _last_in_maps
    _last_in_maps = in_maps
    res = run_bass_kernel_spmd(nc, in_maps, core_ids=list(range(NUM_EXPERTS)))

    out = np.zeros((T, DIM), np.float32)
    for e in range(NUM_EXPERTS):
        yt = np.asarray(res.results[e]["yt"], np.float32)  # [DC, P, C]
        y = yt.reshape(DIM, C)[:, : len(tok[e])].T  # [n_e, D]
        out[tok[e]] += wgt[e][:, None] * y

    return out.reshape(x.shape), aux
